# revision 1
# baseline (speedup 1.0000x reference)
"""Trainium2 Bass kernel for uniform cubic B-spline basis (Cox-de Boor, degree 3).

Math: knots = linspace(-pi, pi, 256) are uniform, so all 252 basis functions are
shifts of one cardinal cubic C(s) supported on [0,4):
    C(s) = (1/6) * sum_k (-1)^k binom(4,k) relu(s-k)^3,  s clamped to [0,4].
For x in knot interval i (u = (x+pi)/h, i = floor(u)), the output row is zero
except columns cstart..cstart+3 with cstart = clip(i-3, 0, 248), and
    row[cstart+m] = C(u - cstart - m).
This handles the edge rows (i<3, i>251) exactly too, because the knot vector is
unclamped uniform.

Per core (131072 rows): rows r = p*1024 + f live in a [128 partitions, 1024]
compute domain. DVE computes u, cstart, flat output element offsets
(r*252 + cstart, int32) and the 4 payload values per row (interleaved
[128, 4*F]). The runtime pre-zeros ExternalOutput buffers (run_bass_kernel_spmd
contract: the native path pre-zeros them, the PJRT path donates zero buffers),
so only the 4 nonzero values per row are written: GPSIMD indirect DMA scatters
16 B per row (one instruction per 128 rows - HW consumes one offset per
partition per instruction, writing each partition's contiguous in_ chunk).
"""
import sys
import types

sys.path.insert(0, "/opt/trn_rl_repo")
sys.path.insert(0, "/root/.axon_site/_ro/trn_rl_repo")

import numpy as np


def _ensure_axon_hooks():
    # antenv.axon_hooks is absent in this image; shim it so trace=True works.
    if "antenv.axon_hooks" in sys.modules:
        return
    try:
        import antenv
    except ImportError:
        return
    m = types.ModuleType("antenv.axon_hooks")
    m._hook = None
    m.set_axon_ntff_profile_hook = lambda h: setattr(m, "_hook", h)
    m.get_axon_ntff_profile_hook = lambda: m._hook
    sys.modules["antenv.axon_hooks"] = m
    antenv.axon_hooks = m
    try:
        from trn_agent_boot.trn_boot import _ntff_profile_via_ctypes

        hook = _ntff_profile_via_ctypes("/opt/axon/libaxon_pjrt.so")
        if hook is not None:
            m.set_axon_ntff_profile_hook(hook)
    except Exception:
        pass


_ensure_axon_hooks()

import concourse.bass as bass
import concourse.mybir as mybir

N = 1_048_576
NCORES = 8
PC = N // NCORES          # rows per core = 131072
P = 128
FTOT = PC // P            # 1024 rows per partition
COLS = 252
NUM_KNOTS = 256

PI = float(np.float32(np.pi))
H = float(np.float32(2.0 * np.pi / (NUM_KNOTS - 1)))
INVH = float(np.float32(1.0) / np.float32(H))
# floor(u) via round-to-nearest(u - 0.5): fold the -0.5 into the bias constant
C1 = float(np.float32(PI - 0.5 * H))

AOT = mybir.AluOpType
F32 = mybir.dt.float32
I32 = mybir.dt.int32

# C(s) = sum_k COEF6[k] * relu(s-k)^3 with the 1/6 folded in
COEF6 = [1.0 / 6.0, -4.0 / 6.0, 1.0, -4.0 / 6.0, 1.0 / 6.0]

# Staggered f-chunk sizes: small first chunks so the GPSIMD scatter (the
# bottleneck, ~1.55 us per 128-row instruction) starts as early as possible.
CHUNKS = [128, 128, 256, 512]
FCMAX = max(CHUNKS)


def build_nc():
    nc = bass.Bass()
    x_in = nc.declare_dram_parameter("x", [PC, 1], F32, isOutput=False)
    out = nc.declare_dram_parameter("out", [PC, COLS], F32, isOutput=True)

    x_flat = x_in[:, 0].rearrange("(p f) -> p f", p=P)  # [128, 1024]
    nchunks = len(CHUNKS)
    starts = [sum(CHUNKS[:i]) for i in range(nchunks)]
    assert sum(CHUNKS) == FTOT

    iota_base = nc.alloc_sbuf_tensor("iota_base", [P, FTOT], I32)
    mneg_i = nc.alloc_sbuf_tensor("mneg_i", [P, 4 * FCMAX], I32)
    mneg = nc.alloc_sbuf_tensor("mneg", [P, 4 * FCMAX], F32)

    with (
        nc.semaphore("isem") as isem,   # iota consts ready
        nc.semaphore("xsem") as xsem,   # x chunk loaded
        nc.semaphore("csem") as csem,   # chunk compute done
        nc.semaphore("dsem") as dsem,   # scatter DMA completions
    ):
        xt = [nc.alloc_sbuf_tensor(f"xt{c}", [P, fc], F32) for c, fc in enumerate(CHUNKS)]
        u = [nc.alloc_sbuf_tensor(f"u{c}", [P, fc], F32) for c, fc in enumerate(CHUNKS)]
        us = [nc.alloc_sbuf_tensor(f"us{c}", [P, fc], F32) for c, fc in enumerate(CHUNKS)]
        ci = nc.alloc_sbuf_tensor("ci", [P, FTOT], I32)
        cst = [nc.alloc_sbuf_tensor(f"cst{c}", [P, fc], F32) for c, fc in enumerate(CHUNKS)]
        d = [nc.alloc_sbuf_tensor(f"d{c}", [P, fc], F32) for c, fc in enumerate(CHUNKS)]
        offi = nc.alloc_sbuf_tensor("offi", [P, FTOT], I32)
        v = [nc.alloc_sbuf_tensor(f"v{c}", [P, 4 * fc], F32) for c, fc in enumerate(CHUNKS)]
        r = [nc.alloc_sbuf_tensor(f"r{c}", [P, 4 * fc], F32) for c, fc in enumerate(CHUNKS)]
        r2 = [nc.alloc_sbuf_tensor(f"r2{c}", [P, 4 * fc], F32) for c, fc in enumerate(CHUNKS)]
        t = [nc.alloc_sbuf_tensor(f"t{c}", [P, 4 * fc], F32) for c, fc in enumerate(CHUNKS)]
        acc = [nc.alloc_sbuf_tensor(f"acc{c}", [P, 4 * fc], F32) for c, fc in enumerate(CHUNKS)]

        with nc.Block() as block:

            @block.sync
            def _(s: bass.BassEngine):
                for ch, fc in enumerate(CHUNKS):
                    s.dma_start(
                        out=xt[ch][:], in_=x_flat[:, starts[ch]: starts[ch] + fc]
                    ).then_inc(xsem, 16)

            @block.gpsimd
            def _(g: bass.BassEngine):
                g.iota(
                    iota_base[:], pattern=[[COLS, FTOT]], base=0,
                    channel_multiplier=FTOT * COLS,
                )
                g.iota(
                    mneg_i[:], pattern=[[0, FCMAX], [-1, 4]], base=0,
                    channel_multiplier=0,
                ).then_inc(isem, 1)
                # int32 adds on GPSIMD: DVE's fp32 ALU would round flat
                # offsets above 2^24 to even, shifting scatters by 1. Each
                # gpsimd tensor op costs ~20us to launch, so do only two:
                # one for chunk 0 (so scatters start early), one for the rest.
                f0 = CHUNKS[0]
                # warm up the Q7 tensor-op ucode (first tensor op after boot
                # costs ~77us; subsequent ones ~2us) while DVE still computes
                g.tensor_tensor(
                    out=offi[:, :2], in0=iota_base[:, :2],
                    in1=iota_base[:, :2], op=AOT.add,
                )
                g.wait_ge(csem, 1)
                g.tensor_tensor(
                    out=offi[:, :f0], in0=iota_base[:, :f0],
                    in1=ci[:, :f0], op=AOT.add,
                )
                for ch, fc in enumerate(CHUNKS):
                    if ch == 1:
                        g.wait_ge(csem, len(CHUNKS))
                        g.tensor_tensor(
                            out=offi[:, f0:], in0=iota_base[:, f0:],
                            in1=ci[:, f0:], op=AOT.add,
                        )
                    for f in range(starts[ch], starts[ch] + fc):
                        g.indirect_dma_start(
                            out=out[:, :],
                            out_offset=bass.IndirectOffsetOnAxis(
                                ap=offi[:, f: f + 1], axis=1
                            ),
                            in_=acc[ch][:, 4 * (f - starts[ch]): 4 * (f - starts[ch]) + 4],
                            in_offset=None,
                        ).then_inc(dsem, 16)
                g.wait_ge(dsem, 16 * FTOT)

            @block.vector
            def _(ve: bass.BassEngine):
                ve.wait_ge(isem, 1)
                ve.tensor_copy(out=mneg[:], in_=mneg_i[:])
                for ch, fc in enumerate(CHUNKS):
                    ve.wait_ge(xsem, 16 * (ch + 1))
                    # u (unshifted) and floor(u) via shifted round-to-nearest
                    ve.tensor_scalar(
                        out=u[ch][:], in0=xt[ch][:], scalar1=PI, scalar2=INVH,
                        op0=AOT.add, op1=AOT.mult,
                    )
                    ve.tensor_scalar(
                        out=us[ch][:], in0=xt[ch][:], scalar1=C1, scalar2=INVH,
                        op0=AOT.add, op1=AOT.mult,
                    )
                    cisl = ci[:, starts[ch]: starts[ch] + fc]
                    ve.tensor_copy(out=cisl, in_=us[ch][:])  # rint = floor(u)
                    ve.tensor_copy(out=cst[ch][:], in_=cisl)
                    # cstart = clip(i-3, 0, 248)
                    ve.tensor_scalar(
                        out=cst[ch][:], in0=cst[ch][:], scalar1=3.0, scalar2=0.0,
                        op0=AOT.subtract, op1=AOT.max,
                    )
                    ve.tensor_scalar_min(
                        out=cst[ch][:], in0=cst[ch][:], scalar1=248.0
                    )
                    # d = u - cstart
                    ve.tensor_tensor(
                        out=d[ch][:], in0=u[ch][:], in1=cst[ch][:],
                        op=AOT.subtract,
                    )
                    # payload v[p, 4f+m] = d - m, clamped at 4
                    ve.tensor_tensor(
                        out=v[ch][:].rearrange("p (f m) -> p f m", m=4),
                        in0=d[ch][:].unsqueeze(2).broadcast_to([P, fc, 4]),
                        in1=mneg[:, : 4 * fc].rearrange("p (f m) -> p f m", m=4),
                        op=AOT.add,
                    )
                    ve.tensor_scalar_min(out=v[ch][:], in0=v[ch][:], scalar1=4.0)
                    for k in range(5):
                        ve.tensor_scalar(
                            out=r[ch][:], in0=v[ch][:], scalar1=float(k),
                            scalar2=0.0, op0=AOT.subtract, op1=AOT.max,
                        )
                        ve.tensor_tensor(
                            out=r2[ch][:], in0=r[ch][:], in1=r[ch][:],
                            op=AOT.mult,
                        )
                        dst = acc[ch] if k == 0 else t[ch]
                        ve.scalar_tensor_tensor(
                            out=dst[:], in0=r2[ch][:], scalar=COEF6[k],
                            in1=r[ch][:], op0=AOT.mult, op1=AOT.mult,
                        )
                        if k > 0:
                            ve.tensor_tensor(
                                out=acc[ch][:], in0=acc[ch][:], in1=t[ch][:],
                                op=AOT.add,
                            )
                    # cstart as int32, last so its completion implies acc is
                    # also final (the offset add happens on GPSIMD)
                    ve.tensor_copy(
                        out=ci[:, starts[ch]: starts[ch] + fc], in_=cst[ch][:]
                    ).then_inc(csem, 1)

    return nc


_CACHED = {}


def kernel(**inputs) -> np.ndarray:
    from concourse.bass_utils import run_bass_kernel_spmd

    x = np.asarray(inputs["x"], dtype=np.float32).reshape(N, 1)
    if "nc" not in _CACHED:
        _CACHED["nc"] = build_nc()
    nc = _CACHED["nc"]
    in_maps = [{"x": x[c * PC: (c + 1) * PC]} for c in range(NCORES)]
    res = run_bass_kernel_spmd(nc, in_maps, list(range(NCORES)))
    return np.concatenate([r["out"] for r in res.results], axis=0)


if __name__ == "__main__":
    rng = np.random.default_rng(0)
    xs = rng.uniform(-np.pi, np.pi, size=(N, 1)).astype(np.float32)
    o = kernel(x=xs)
    print("out", o.shape, o.dtype, float(np.abs(o).max()))



# revision 30
# speedup vs baseline: 1.3441x; 1.3441x over previous
"""Trainium2 Bass kernel for uniform cubic B-spline basis (Cox-de Boor, degree 3).

Uniform knots => all 252 basis functions are shifts of one cardinal cubic C(s)
on [0,4). Row r is zero except columns cstart..cstart+3 (cstart = clip(i-3, 0,
248), i = floor(u), u = (x+pi)/h), holding C(u-cstart-m).

Strategy: batch the output scatter with the custom-ucode `dma_scatter_add`
(out[idxs,:] += in, 2048 rows per SWDGE instruction) instead of one GPSIMD
indirect DMA per 128 rows (994ns fixed overhead x 1024 = the old 1.6ms).
Each row writes one 68-float window at a 256B-aligned address; the window is
computed DENSELY as win[j] = C(w - j), w = (u - cstart) + o, via
    y = |s-2|, C = relu(2-y)^3/6 - (2/3)*relu(1-y)^3   (s outside [0,4] -> 0)
split across DVE and Act. Only full-tensor ops are used on the compute
engines: Bacc's dependency tracker does not model strided-AP overlaps, and
its compile passes will reorder engine instructions whose dependency it
cannot see (found the hard way).

Output layout: DRAM [PC+1, 256] f32, column c of row r at element 256r+4+c.
The +4 shift and the pad row make every window's spill land in pad slots that
only ever receive zero-adds, so concurrent CCE adds never race on real data.
Host slices [:PC, 4:256]. Rows map partition-minor (row r on partition r%128)
so scatter token i == row i. idx = 4*(r%2048... per-prep) + (cstart+4)>>6 and
w are computed on the host in float32 and uploaded; the int16 idx tensor is
16-partition-wrapped and replicated. The runtime pre-zeros ExternalOutput
buffers, so scatter-add acts as scatter-write.
"""
import sys
import types

sys.path.insert(0, "/opt/trn_rl_repo")
sys.path.insert(0, "/root/.axon_site/_ro/trn_rl_repo")

import numpy as np


def _ensure_axon_hooks():
    if "antenv.axon_hooks" in sys.modules:
        return
    try:
        import antenv
    except ImportError:
        return
    m = types.ModuleType("antenv.axon_hooks")
    m._hook = None
    m.set_axon_ntff_profile_hook = lambda h: setattr(m, "_hook", h)
    m.get_axon_ntff_profile_hook = lambda: m._hook
    sys.modules["antenv.axon_hooks"] = m
    antenv.axon_hooks = m
    try:
        from trn_agent_boot.trn_boot import _ntff_profile_via_ctypes

        hook = _ntff_profile_via_ctypes("/opt/axon/libaxon_pjrt.so")
        if hook is not None:
            m.set_axon_ntff_profile_hook(hook)
    except Exception:
        pass


_ensure_axon_hooks()

import concourse.bass as bass
import concourse.bacc as bacc
import concourse.mybir as mybir
from concourse.library_config import mlp as mlp_lib

N = 1_048_576
NCORES = 8
PC = N // NCORES          # 131072 rows per core
P = 128
F = PC // P               # 1024 slots per partition
COLS = 252
OC = 256
NUM_KNOTS = 256

PI = float(np.float32(np.pi))
H = float(np.float32(2.0 * np.pi / (NUM_KNOTS - 1)))
INVH = float(np.float32(1.0) / np.float32(H))
C1 = float(np.float32(PI - 0.5 * H))

AOT = mybir.AluOpType
AFT = mybir.ActivationFunctionType
F32 = mybir.dt.float32
I16 = mybir.dt.int16

FC = 32                   # slots per chunk -> 4096 rows
NCHUNK = F // FC          # 32
PREP = 2048               # tokens per scatter instruction (2 per chunk)
WIN = 68
STEP = 64                 # 256B idx granularity
XDST = 16387
NWBUF = 3


def build_nc():
    nc = bacc.Bacc("TRN2", dynamic_dma_scratch_size=65536)
    w_d = nc.declare_dram_parameter("w", [P, F], F32, isOutput=False)
    ix_d = nc.declare_dram_parameter("idxw", [P, F * 8], I16, isOutput=False)
    out = nc.declare_dram_parameter("out", [PC + 1, OC], F32, isOutput=True)

    w = nc.alloc_sbuf_tensor("w_s", [P, F], F32)
    idxw = nc.alloc_sbuf_tensor("idxw_s", [P, F * 8], I16)
    iotaf = nc.alloc_sbuf_tensor("iotaf", [P, FC * WIN], F32)
    warm_idx = nc.alloc_sbuf_tensor("warm_idx", [P, 1], I16)
    warm_src = nc.alloc_sbuf_tensor("warm_src", [P, WIN], F32)
    bias_m2 = nc.alloc_sbuf_tensor("bias_m2", [P, 1], F32)
    bias_p2 = nc.alloc_sbuf_tensor("bias_p2", [P, 1], F32)
    bias_p1 = nc.alloc_sbuf_tensor("bias_p1", [P, 1], F32)

    sb = [nc.alloc_sbuf_tensor(f"sb{i}", [P, FC * WIN], F32) for i in range(2)]
    zb = [nc.alloc_sbuf_tensor(f"zb{i}", [P, FC * WIN], F32) for i in range(2)]
    ub = [nc.alloc_sbuf_tensor(f"ub{i}", [P, FC * WIN], F32) for i in range(2)]
    zq = nc.alloc_sbuf_tensor("zq", [P, FC * WIN], F32)
    win = [nc.alloc_sbuf_tensor(f"win{i}", [P, FC * WIN], F32) for i in range(NWBUF)]

    with (
        nc.semaphore("insem") as insem,
        nc.semaphore("gsem") as gsem,
        nc.semaphore("sA") as sA,      # DVE s-grid ready
        nc.semaphore("sB") as sB,      # act y/z/u ready
        nc.semaphore("sC") as sC,      # DVE consumed z/u bufs
        nc.semaphore("csem") as csem,  # win content final
        nc.semaphore("prepsem") as prepsem,
        nc.semaphore("dsb0") as dsb0,  # per-win-buffer transfer completion
        nc.semaphore("dsb1") as dsb1,
        nc.semaphore("dsb2") as dsb2,
    ):
        dsb = [dsb0, dsb1, dsb2]
        with nc.Block() as block:

            @block.sync
            def _(s: bass.BassEngine):
                s.dma_start(out=w[:], in_=w_d[:, :]).then_inc(insem, 16)
                s.dma_start(out=idxw[:], in_=ix_d[:, :]).then_inc(insem, 16)

            @block.gpsimd
            def _(g: bass.BassEngine):
                g.iota(warm_idx[:], pattern=[[0, 1]], base=0,
                       channel_multiplier=0)
                g.memset(warm_src[:], 0.0)
                g.memset(bias_m2[:], -2.0)
                g.memset(bias_p2[:], 2.0)
                g.memset(bias_p1[:], 1.0)
                # j-grid 0..67 repeated per slot; f32 ints < 2^24 are exact
                g.iota(iotaf[:], pattern=[[0, FC], [1, WIN]], base=0,
                       channel_multiplier=0,
                       allow_small_or_imprecise_dtypes=True).then_inc(gsem, 1)
                g.wait_ge(gsem, 1)
                g.load_library(mlp_lib)
                # warm the scatter ucode: 16 zero-value descs into the pad row
                g.dma_scatter_add(
                    out_ap=bass.AP(out, PC * OC, [[STEP, 3], [1, WIN]]),
                    in_ap=warm_src[:, :].rearrange("p (s e) -> p s e", e=WIN),
                    idxs_ap=warm_idx[:, :],
                    num_idxs=16, num_idxs_reg=16,
                    elem_size=WIN, elem_step=STEP,
                    prepare_only=True, sem=dsb[0],
                ).then_inc(prepsem, 1)
                g.wait_ge(prepsem, 1)
                g.trigger_dma(count=1)

                g.wait_ge(insem, 32)
                prep_reg = g.to_reg(PREP)
                for c in range(NCHUNK):
                    g.wait_ge(csem, c + 1)
                    base = c * (FC * P) * OC
                    for half in range(2):
                        hs = half * (PREP // P)
                        g.dma_scatter_add(
                            out_ap=bass.AP(out, base, [[STEP, XDST], [1, WIN]]),
                            in_ap=win[c % NWBUF][
                                :, hs * WIN: (hs + PREP // P) * WIN
                            ].rearrange("p (s e) -> p s e", e=WIN),
                            idxs_ap=idxw[
                                :, c * (FC * P // 16) + half * (PREP // 16):
                                c * (FC * P // 16) + (half + 1) * (PREP // 16)
                            ],
                            num_idxs=PREP, num_idxs_reg=prep_reg,
                            elem_size=WIN, elem_step=STEP,
                            prepare_only=True, sem=dsb[c % NWBUF],
                        ).then_inc(prepsem, 1)
                    g.wait_ge(prepsem, 1 + 2 * (c + 1))
                    g.trigger_dma(count=2)
                for b in range(NWBUF):
                    uses = len([c for c in range(NCHUNK) if c % NWBUF == b])
                    g.wait_ge(dsb[b], 32 * uses + (16 if b == 0 else 0))

            @block.scalar
            def _(a: bass.BassEngine):
                a.wait_ge(gsem, 1)  # bias tensors ready
                for c in range(NCHUNK):
                    a.wait_ge(sA, c + 1)
                    if c >= 2:
                        a.wait_ge(sC, c - 1)  # zb/ub[c%2] free
                    sbc, zbc, ubc = sb[c % 2], zb[c % 2], ub[c % 2]
                    # y = |s - 2| (in place), z = relu(2-y), u = relu(1-y)
                    a.activation(out=sbc[:], in_=sbc[:], func=AFT.Abs,
                                 bias=bias_m2[:, :])
                    a.activation(out=zbc[:], in_=sbc[:], func=AFT.Relu,
                                 bias=bias_p2[:, :], scale=-1.0)
                    last = a.activation(out=ubc[:], in_=sbc[:], func=AFT.Relu,
                                        bias=bias_p1[:, :], scale=-1.0)
                    last.then_inc(sB, 1)

            @block.vector
            def _(ve: bass.BassEngine):
                ve.wait_ge(gsem, 1)
                ve.wait_ge(insem, 32)
                for c in range(NCHUNK):
                    if c >= 2:
                        ve.wait_ge(sB, c - 1)  # sb[c%2] consumed by act
                    # s = w - j over the 68-wide grid
                    ve.tensor_tensor(
                        out=sb[c % 2][:, :].rearrange("p (s e) -> p s e", e=WIN),
                        in0=w[:, c * FC:(c + 1) * FC].unsqueeze(2)
                        .broadcast_to([P, FC, WIN]),
                        in1=iotaf[:, :].rearrange("p (s e) -> p s e", e=WIN),
                        op=AOT.subtract,
                    ).then_inc(sA, 1)
                    ve.wait_ge(sB, c + 1)
                    wb = win[c % NWBUF]
                    b = c % NWBUF
                    need = 32 * (c // NWBUF) + (16 if b == 0 else 0)
                    if need:
                        ve.wait_ge(dsb[b], need)  # win buffer drained
                    zbc, ubc = zb[c % 2], ub[c % 2]
                    # win = relu(2-y)^3/6
                    ve.tensor_tensor(out=zq[:], in0=zbc[:], in1=zbc[:],
                                     op=AOT.mult)
                    ve.scalar_tensor_tensor(out=wb[:], in0=zq[:],
                                            scalar=1.0 / 6.0, in1=zbc[:],
                                            op0=AOT.mult, op1=AOT.mult)
                    # win -= (2/3) relu(1-y)^3
                    ve.tensor_tensor(out=zq[:], in0=ubc[:], in1=ubc[:],
                                     op=AOT.mult)
                    ve.scalar_tensor_tensor(
                        out=zq[:], in0=zq[:], scalar=2.0 / 3.0, in1=ubc[:],
                        op0=AOT.mult, op1=AOT.mult,
                    ).then_inc(sC, 1)  # zb/ub consumed
                    ve.tensor_tensor(out=wb[:], in0=wb[:], in1=zq[:],
                                     op=AOT.subtract).then_inc(csem, 1)

    nc.compile()
    return nc


_CACHED = {}


def make_in_maps(x: np.ndarray) -> list[dict]:
    xs = np.ascontiguousarray(np.asarray(x).reshape(N).astype(np.float32))
    u = (xs + np.float32(PI)) * np.float32(INVH)
    us = (xs + np.float32(C1)) * np.float32(INVH)
    ci = np.rint(us).astype(np.int64)          # == floor(u)
    cst = np.clip(ci - 3, 0, 248)
    dd = u - cst.astype(np.float32)
    e = cst + 4
    k = e >> 6
    o = (e & 63).astype(np.float32)
    wv = dd + o                                # win[j] = C(w - j)
    maps = []
    for c in range(NCORES):
        s = slice(c * PC, (c + 1) * PC)
        r_loc = np.arange(PC, dtype=np.int64)
        idxv = (4 * (r_loc % (FC * P)) + k[s]).astype(np.int16)
        wrapped = np.tile(
            np.ascontiguousarray(idxv.reshape(PC // 16, 16).T), (8, 1)
        )
        maps.append({
            "w": np.ascontiguousarray(wv[s].reshape(F, P).T),
            "idxw": np.ascontiguousarray(wrapped),
        })
    return maps


def kernel(**inputs) -> np.ndarray:
    from concourse.bass_utils import run_bass_kernel_spmd

    x = np.asarray(inputs["x"], dtype=np.float32).reshape(N, 1)
    if "nc" not in _CACHED:
        _CACHED["nc"] = build_nc()
    nc = _CACHED["nc"]
    in_maps = make_in_maps(x)
    res = run_bass_kernel_spmd(nc, in_maps, list(range(NCORES)))
    return np.concatenate(
        [np.ascontiguousarray(r["out"][:PC, 4: 4 + COLS]) for r in res.results],
        axis=0,
    )


if __name__ == "__main__":
    rng = np.random.default_rng(0)
    xs = rng.uniform(-np.pi, np.pi, size=(N, 1)).astype(np.float32)
    o = kernel(x=xs)
    print("out", o.shape, o.dtype, float(np.abs(o).max()))


# revision 31
# speedup vs baseline: 1.5111x; 1.1243x over previous
"""Trainium2 Bass kernel for uniform cubic B-spline basis (Cox-de Boor, degree 3).

Uniform knots => all 252 basis functions are shifts of one cardinal cubic C(s)
on [0,4). Row r is zero except columns cstart..cstart+3 (cstart = clip(i-3, 0,
248), i = floor(u), u = (x+pi)/h), holding C(u-cstart-m).

Strategy: batch the output scatter with the custom-ucode `dma_scatter_add`
(out[idxs,:] += in, 2048 rows per SWDGE instruction) instead of one GPSIMD
indirect DMA per 128 rows (994ns fixed overhead x 1024 = the old 1.6ms).
Each row writes one 68-float window at a 256B-aligned address; the window is
computed DENSELY as win[j] = C(w - j), w = (u - cstart) + o, via
    y = |s-2|, C = relu(2-y)^3/6 - (2/3)*relu(1-y)^3   (s outside [0,4] -> 0)
split across DVE and Act. Only full-tensor ops are used on the compute
engines: Bacc's dependency tracker does not model strided-AP overlaps, and
its compile passes will reorder engine instructions whose dependency it
cannot see (found the hard way).

Output layout: DRAM [PC+1, 256] f32, column c of row r at element 256r+4+c.
The +4 shift and the pad row make every window's spill land in pad slots that
only ever receive zero-adds, so concurrent CCE adds never race on real data.
Host slices [:PC, 4:256]. Rows map partition-minor (row r on partition r%128)
so scatter token i == row i. idx = 4*(r%2048... per-prep) + (cstart+4)>>6 and
w are computed on the host in float32 and uploaded; the int16 idx tensor is
16-partition-wrapped and replicated. The runtime pre-zeros ExternalOutput
buffers, so scatter-add acts as scatter-write.
"""
import sys
import types

sys.path.insert(0, "/opt/trn_rl_repo")
sys.path.insert(0, "/root/.axon_site/_ro/trn_rl_repo")

import numpy as np


def _ensure_axon_hooks():
    if "antenv.axon_hooks" in sys.modules:
        return
    try:
        import antenv
    except ImportError:
        return
    m = types.ModuleType("antenv.axon_hooks")
    m._hook = None
    m.set_axon_ntff_profile_hook = lambda h: setattr(m, "_hook", h)
    m.get_axon_ntff_profile_hook = lambda: m._hook
    sys.modules["antenv.axon_hooks"] = m
    antenv.axon_hooks = m
    try:
        from trn_agent_boot.trn_boot import _ntff_profile_via_ctypes

        hook = _ntff_profile_via_ctypes("/opt/axon/libaxon_pjrt.so")
        if hook is not None:
            m.set_axon_ntff_profile_hook(hook)
    except Exception:
        pass


_ensure_axon_hooks()

import concourse.bass as bass
import concourse.bacc as bacc
import concourse.mybir as mybir
from concourse.library_config import mlp as mlp_lib

N = 1_048_576
NCORES = 8
PC = N // NCORES          # 131072 rows per core
P = 128
F = PC // P               # 1024 slots per partition
COLS = 252
OC = 256
NUM_KNOTS = 256

PI = float(np.float32(np.pi))
H = float(np.float32(2.0 * np.pi / (NUM_KNOTS - 1)))
INVH = float(np.float32(1.0) / np.float32(H))
C1 = float(np.float32(PI - 0.5 * H))

AOT = mybir.AluOpType
AFT = mybir.ActivationFunctionType
F32 = mybir.dt.float32
I16 = mybir.dt.int16

FC = 32                   # slots per chunk -> 4096 rows
NCHUNK = F // FC          # 32
PREP = 2048               # tokens per scatter instruction (2 per chunk)
WIN = 68
STEP = 64                 # 256B idx granularity
XDST = 16387
NWBUF = 3


def build_nc():
    nc = bacc.Bacc("TRN2", dynamic_dma_scratch_size=65536)
    w_d = nc.declare_dram_parameter("w", [P, F], F32, isOutput=False)
    ix_d = nc.declare_dram_parameter("idxw", [P, F * 8], I16, isOutput=False)
    out = nc.declare_dram_parameter("out", [PC + 1, OC], F32, isOutput=True)

    w = nc.alloc_sbuf_tensor("w_s", [P, F], F32)
    idxw = nc.alloc_sbuf_tensor("idxw_s", [P, F * 8], I16)
    iotaf = nc.alloc_sbuf_tensor("iotaf", [P, FC * WIN], F32)
    warm_idx = nc.alloc_sbuf_tensor("warm_idx", [P, 1], I16)
    warm_src = nc.alloc_sbuf_tensor("warm_src", [P, WIN], F32)
    bias_m2 = nc.alloc_sbuf_tensor("bias_m2", [P, 1], F32)
    bias_p2 = nc.alloc_sbuf_tensor("bias_p2", [P, 1], F32)
    bias_p1 = nc.alloc_sbuf_tensor("bias_p1", [P, 1], F32)

    sb = [nc.alloc_sbuf_tensor(f"sb{i}", [P, FC * WIN], F32) for i in range(2)]
    zb = [nc.alloc_sbuf_tensor(f"zb{i}", [P, FC * WIN], F32) for i in range(2)]
    ub = [nc.alloc_sbuf_tensor(f"ub{i}", [P, FC * WIN], F32) for i in range(2)]
    zq = nc.alloc_sbuf_tensor("zq", [P, FC * WIN], F32)
    win = [nc.alloc_sbuf_tensor(f"win{i}", [P, FC * WIN], F32) for i in range(NWBUF)]

    with (
        nc.semaphore("insem") as insem,
        nc.semaphore("gsem") as gsem,
        nc.semaphore("sA") as sA,      # DVE s-grid ready
        nc.semaphore("sB") as sB,      # act y/z/u ready
        nc.semaphore("sC") as sC,      # DVE consumed z/u bufs
        nc.semaphore("csem") as csem,  # win content final
        nc.semaphore("prepsem") as prepsem,
        nc.semaphore("dsb0") as dsb0,  # per-win-buffer transfer completion
        nc.semaphore("dsb1") as dsb1,
        nc.semaphore("dsb2") as dsb2,
    ):
        dsb = [dsb0, dsb1, dsb2]
        with nc.Block() as block:

            @block.sync
            def _(s: bass.BassEngine):
                s.dma_start(out=w[:], in_=w_d[:, :]).then_inc(insem, 16)
                s.dma_start(out=idxw[:], in_=ix_d[:, :]).then_inc(insem, 16)

            @block.gpsimd
            def _(g: bass.BassEngine):
                g.iota(warm_idx[:], pattern=[[0, 1]], base=0,
                       channel_multiplier=0)
                g.memset(warm_src[:], 0.0)
                g.memset(bias_m2[:], -2.0)
                g.memset(bias_p2[:], 2.0)
                g.memset(bias_p1[:], 1.0)
                # j-grid 0..67 repeated per slot; f32 ints < 2^24 are exact
                g.iota(iotaf[:], pattern=[[0, FC], [1, WIN]], base=0,
                       channel_multiplier=0,
                       allow_small_or_imprecise_dtypes=True).then_inc(gsem, 1)
                g.wait_ge(gsem, 1)
                g.load_library(mlp_lib)
                # warm the scatter ucode: 16 zero-value descs into the pad row
                g.dma_scatter_add(
                    out_ap=bass.AP(out, PC * OC, [[STEP, 3], [1, WIN]]),
                    in_ap=warm_src[:, :].rearrange("p (s e) -> p s e", e=WIN),
                    idxs_ap=warm_idx[:, :],
                    num_idxs=16, num_idxs_reg=16,
                    elem_size=WIN, elem_step=STEP,
                    prepare_only=True, sem=dsb[0],
                ).then_inc(prepsem, 1)
                g.wait_ge(prepsem, 1)
                g.trigger_dma(count=1)

                g.wait_ge(insem, 32)
                prep_reg = g.to_reg(PREP)
                for c in range(NCHUNK):
                    g.wait_ge(csem, c + 1)
                    base = c * (FC * P) * OC
                    for half in range(2):
                        hs = half * (PREP // P)
                        g.dma_scatter_add(
                            out_ap=bass.AP(out, base, [[STEP, XDST], [1, WIN]]),
                            in_ap=win[c % NWBUF][
                                :, hs * WIN: (hs + PREP // P) * WIN
                            ].rearrange("p (s e) -> p s e", e=WIN),
                            idxs_ap=idxw[
                                :, c * (FC * P // 16) + half * (PREP // 16):
                                c * (FC * P // 16) + (half + 1) * (PREP // 16)
                            ],
                            num_idxs=PREP, num_idxs_reg=prep_reg,
                            elem_size=WIN, elem_step=STEP,
                            prepare_only=True, sem=dsb[c % NWBUF],
                        ).then_inc(prepsem, 1)
                    g.wait_ge(prepsem, 1 + 2 * (c + 1))
                    g.trigger_dma(count=2)
                for b in range(NWBUF):
                    uses = len([c for c in range(NCHUNK) if c % NWBUF == b])
                    g.wait_ge(dsb[b], 32 * uses + (16 if b == 0 else 0))

            @block.scalar
            def _(a: bass.BassEngine):
                a.wait_ge(gsem, 1)  # bias tensors ready
                for c in range(NCHUNK):
                    a.wait_ge(sA, c + 1)
                    if c >= 2:
                        a.wait_ge(sC, c - 1)  # zb/ub[c%2] free
                    sbc, zbc, ubc = sb[c % 2], zb[c % 2], ub[c % 2]
                    # y = |s - 2| (in place), z = relu(2-y), u = relu(1-y)
                    a.activation(out=sbc[:], in_=sbc[:], func=AFT.Abs,
                                 bias=bias_m2[:, :])
                    a.activation(out=zbc[:], in_=sbc[:], func=AFT.Relu,
                                 bias=bias_p2[:, :], scale=-1.0)
                    last = a.activation(out=ubc[:], in_=sbc[:], func=AFT.Relu,
                                        bias=bias_p1[:, :], scale=-1.0)
                    last.then_inc(sB, 1)

            @block.vector
            def _(ve: bass.BassEngine):
                ve.wait_ge(gsem, 1)
                ve.wait_ge(insem, 32)

                def s_grid(c):
                    # s = w - j over the 68-wide grid
                    ve.tensor_tensor(
                        out=sb[c % 2][:, :].rearrange("p (s e) -> p s e", e=WIN),
                        in0=w[:, c * FC:(c + 1) * FC].unsqueeze(2)
                        .broadcast_to([P, FC, WIN]),
                        in1=iotaf[:, :].rearrange("p (s e) -> p s e", e=WIN),
                        op=AOT.subtract,
                    ).then_inc(sA, 1)

                # software pipeline: compute chunk c+2's s-grid while act
                # processes c+1 and DVE finishes c.
                s_grid(0)
                s_grid(1)
                for c in range(NCHUNK):
                    ve.wait_ge(sB, c + 1)
                    if c + 2 < NCHUNK:
                        s_grid(c + 2)  # sb[c%2] free: act(c) just finished it
                    wb = win[c % NWBUF]
                    b = c % NWBUF
                    need = 32 * (c // NWBUF) + (16 if b == 0 else 0)
                    if need:
                        ve.wait_ge(dsb[b], need)  # win buffer drained
                    zbc, ubc = zb[c % 2], ub[c % 2]
                    # win = relu(2-y)^3/6
                    ve.tensor_tensor(out=zq[:], in0=zbc[:], in1=zbc[:],
                                     op=AOT.mult)
                    ve.scalar_tensor_tensor(out=wb[:], in0=zq[:],
                                            scalar=1.0 / 6.0, in1=zbc[:],
                                            op0=AOT.mult, op1=AOT.mult)
                    # win -= (2/3) relu(1-y)^3
                    ve.tensor_tensor(out=zq[:], in0=ubc[:], in1=ubc[:],
                                     op=AOT.mult)
                    ve.scalar_tensor_tensor(
                        out=zq[:], in0=zq[:], scalar=2.0 / 3.0, in1=ubc[:],
                        op0=AOT.mult, op1=AOT.mult,
                    ).then_inc(sC, 1)  # zb/ub consumed
                    ve.tensor_tensor(out=wb[:], in0=wb[:], in1=zq[:],
                                     op=AOT.subtract).then_inc(csem, 1)

    nc.compile()
    return nc


_CACHED = {}


def make_in_maps(x: np.ndarray) -> list[dict]:
    xs = np.ascontiguousarray(np.asarray(x).reshape(N).astype(np.float32))
    u = (xs + np.float32(PI)) * np.float32(INVH)
    us = (xs + np.float32(C1)) * np.float32(INVH)
    ci = np.rint(us).astype(np.int64)          # == floor(u)
    cst = np.clip(ci - 3, 0, 248)
    dd = u - cst.astype(np.float32)
    e = cst + 4
    k = e >> 6
    o = (e & 63).astype(np.float32)
    wv = dd + o                                # win[j] = C(w - j)
    maps = []
    for c in range(NCORES):
        s = slice(c * PC, (c + 1) * PC)
        r_loc = np.arange(PC, dtype=np.int64)
        idxv = (4 * (r_loc % (FC * P)) + k[s]).astype(np.int16)
        wrapped = np.tile(
            np.ascontiguousarray(idxv.reshape(PC // 16, 16).T), (8, 1)
        )
        maps.append({
            "w": np.ascontiguousarray(wv[s].reshape(F, P).T),
            "idxw": np.ascontiguousarray(wrapped),
        })
    return maps


def kernel(**inputs) -> np.ndarray:
    from concourse.bass_utils import run_bass_kernel_spmd

    x = np.asarray(inputs["x"], dtype=np.float32).reshape(N, 1)
    if "nc" not in _CACHED:
        _CACHED["nc"] = build_nc()
    nc = _CACHED["nc"]
    in_maps = make_in_maps(x)
    res = run_bass_kernel_spmd(nc, in_maps, list(range(NCORES)))
    return np.concatenate(
        [np.ascontiguousarray(r["out"][:PC, 4: 4 + COLS]) for r in res.results],
        axis=0,
    )


if __name__ == "__main__":
    rng = np.random.default_rng(0)
    xs = rng.uniform(-np.pi, np.pi, size=(N, 1)).astype(np.float32)
    o = kernel(x=xs)
    print("out", o.shape, o.dtype, float(np.abs(o).max()))
